# revision 32
# baseline (speedup 1.0000x reference)
"""Self-attention scores kernel for Trainium2, 8-core SPMD.

Computes softmax((x@Wq+bq) @ (x@Wq+bq)^T / sqrt(64)) per head
(reference reuses the query projection for k, bug-for-bug).

Sharding: 32 (batch, head) pairs split 4-per-core across 8 cores.
Core c handles batch c//4, heads 4*(c%4) .. 4*(c%4)+3.

Per core: project q^T = Wq_slice^T @ x^T (+bias) from fp16 inputs into
a bf16 q^T tile, then per head stream [128, 2048] score row-blocks:
4 matmuls into a 4-bank PSUM tile, one Exp activation (N=2048, f32
accum row-sum) into a bf16 SBUF tile, reciprocal + per-row scale on
DVE, and bf16 DMA back to HBM (2 MiB groups; the last head drains
per-block, alternating the sync/gpsimd rings). Host upcasts bf16->f32.

bf16 output halves the dominant HBM write traffic vs f32; softmax
values are <= 1 so bf16's 7-bit mantissa costs ~3e-3 relative error,
well within tolerance.
"""

import numpy as np

import concourse.bass as bass
import concourse.mybir as mybir
import concourse.tile as tile
from concourse import bacc
from concourse.bass_utils import run_bass_kernel_spmd

B = 2
S = 2048
D = 1024
H = 16
HS = 64
N_CORES = 8
HEADS_PER_CORE = 4  # 2 pairs of 2 heads (pair = 128 partitions)
KK = D // 128  # 8 k-tiles for the projection contraction
NQ = S // 128  # 16 q row-blocks per head
GRP = 4  # row-blocks batched per output DMA (2 MiB)

# Matmul operand dtype for q^T: bf16 runs at full PE rate and halves
# the per-block LDWEIGHTS time in the PSUM-handoff critical loop vs
# fp32r; its 8-bit mantissa adds ~3e-4 rms to the logits, well inside
# tolerance.
MM_DT = mybir.dt.bfloat16
# Projection inputs in fp16: 11-bit mantissa matches fp32r precision
# while halving the input-load bytes.
IN_DT = mybir.dt.float16
# Output path in bf16: softmax values are <= 1; halves HBM writes.
OUT_DT = mybir.dt.bfloat16

F32 = mybir.dt.float32


def _build():
    nc = bacc.Bacc("TRN2", target_bir_lowering=False, debug=False)
    xT = nc.dram_tensor("xT", [D, S], IN_DT, kind="ExternalInput").ap()
    WqS = nc.dram_tensor("WqS", [D, HEADS_PER_CORE * HS], IN_DT, kind="ExternalInput").ap()
    bqS = nc.dram_tensor("bqS", [128, 2], F32, kind="ExternalInput").ap()
    out = nc.dram_tensor("out", [HEADS_PER_CORE, S, S], OUT_DT, kind="ExternalOutput").ap()

    with tile.TileContext(nc) as tc:
        with (
            tc.tile_pool(name="consts", bufs=1) as consts,
            tc.tile_pool(name="xt", bufs=KK) as xt_pool,
            tc.tile_pool(name="ps", bufs=2, space="PSUM") as ps_pool,
            tc.tile_pool(name="et", bufs=3) as et_pool,
            tc.tile_pool(name="small", bufs=4) as small,
        ):
            w = consts.tile([128, KK, HEADS_PER_CORE * HS], IN_DT)
            nc.scalar.dma_start(out=w[:], in_=WqS.rearrange("(kk p) c -> p kk c", p=128))
            bias = consts.tile([128, 2], F32)
            nc.scalar.dma_start(out=bias[:], in_=bqS)

            # x^T streamed as 8 independent k-tiles so projection matmuls
            # can start as soon as each tile lands.
            xts = []
            for kk in range(KK):
                xtt = xt_pool.tile([128, S], IN_DT, tag="xt")
                nc.sync.dma_start(out=xtt[:], in_=xT[kk * 128 : (kk + 1) * 128, :])
                xts.append(xtt)

            # ---- Projection for one head-pair: fills a full 4-bank PSUM
            # tile (4 x 512-col accumulation groups of 8 matmuls), then one
            # bias-add into the persistent q^T tile.
            # Both pairs' projections interleaved kk-outer: PE streams the
            # 8-matmul kk groups as each x^T tile lands, so the projection
            # finishes right after the input load instead of serializing
            # 64 matmuls after it.
            # Pair 0 fully first (kk-outer, streaming as x^T tiles land),
            # then pair 1: qt0 and its bias-add complete ~7us earlier, and
            # pair 1's matmuls overlap the bias-add and the first score
            # fills instead of delaying them.
            qts = []
            for g in range(2):
                psg = ps_pool.tile([128, S], F32, tag="ps", name=f"ps{g}")
                for kk in range(KK):
                    for n in range(4):
                        nc.tensor.matmul(
                            psg[:, n * 512 : (n + 1) * 512],
                            lhsT=w[:, kk, g * 128 : (g + 1) * 128],
                            rhs=xts[kk][:, n * 512 : (n + 1) * 512],
                            start=(kk == 0),
                            stop=(kk == KK - 1),
                        )
                qtg = consts.tile([128, S], MM_DT, tag=f"qt{g}", name=f"qt{g}")
                nc.vector.tensor_scalar_add(qtg[:], psg[:], bias[:, g : g + 1])
                qts.append(qtg)

            # ---- Scores + softmax, streamed in groups of 4 row-blocks ----
            # The last head drains per-block (512 KiB DMAs right after each
            # row-scale) so the kernel tail trails the final exp by one
            # block; earlier heads use 2 MiB group DMAs. Output DMAs
            # alternate between the HWDGE (sync) and SWDGE (gpsimd) rings —
            # the Pool engine is otherwise idle, and the second ring adds
            # queue parallelism to the write stream.
            for h in range(HEADS_PER_CORE):
                qtg = qts[h // 2]
                pb = (h % 2) * 64
                for grp in range(NQ // GRP):
                    last_grp = h == HEADS_PER_CORE - 1
                    et = et_pool.tile([128, GRP, S], OUT_DT, tag="et")
                    sums = small.tile([128, GRP], F32, tag="sm")
                    rec = small.tile([128, GRP], F32, tag="rc")
                    for q in range(GRP):
                        i = grp * GRP + q
                        ps = ps_pool.tile([128, S], F32, tag="ps")
                        lhsT = qtg[pb : pb + 64, i * 128 : (i + 1) * 128]
                        for j in range(4):
                            nc.tensor.matmul(
                                ps[:, j * 512 : (j + 1) * 512],
                                lhsT=lhsT,
                                rhs=qtg[pb : pb + 64, j * 512 : (j + 1) * 512],
                                start=True,
                                stop=True,
                            )
                        nc.scalar.activation(
                            out=et[:, q, :],
                            in_=ps[:],
                            func=mybir.ActivationFunctionType.Exp,
                            scale=1.0 / np.sqrt(float(HS)),
                            accum_out=sums[:, q : q + 1],
                        )
                        if last_grp:
                            nc.vector.reciprocal(rec[:, q : q + 1], sums[:, q : q + 1])
                            nc.vector.tensor_scalar_mul(
                                et[:, q, :], et[:, q, :], rec[:, q : q + 1]
                            )
                            eng = nc.sync if i % 2 == 0 else nc.gpsimd
                            eng.dma_start(
                                out=out[h, i * 128 : (i + 1) * 128, :],
                                in_=et[:, q, :],
                            )
                    if last_grp:
                        continue
                    nc.vector.reciprocal(rec[:], sums[:])
                    for q in range(GRP):
                        nc.vector.tensor_scalar_mul(
                            et[:, q, :], et[:, q, :], rec[:, q : q + 1]
                        )
                    eng = nc.sync if grp % 2 == 0 else nc.gpsimd
                    eng.dma_start(
                        out=out[h, grp * GRP * 128 : (grp + 1) * GRP * 128, :].rearrange(
                            "(c p) s -> p c s", p=128
                        ),
                        in_=et[:],
                    )
    nc.compile()
    return nc


_NC_CACHE = None


def kernel(x, Wq, bq):
    global _NC_CACHE
    x = np.asarray(x, dtype=np.float32)
    Wq = np.asarray(Wq, dtype=np.float32)
    bq = np.asarray(bq, dtype=np.float32)
    assert x.shape == (B, S, D) and Wq.shape == (D, D) and bq.shape == (D,)

    if _NC_CACHE is None:
        _NC_CACHE = _build()
    nc = _NC_CACHE

    xTs = [np.ascontiguousarray(x[b].T.astype(np.float16)) for b in range(B)]
    Wq16 = Wq.astype(np.float16)
    in_maps = []
    for c in range(N_CORES):
        b, hg = divmod(c, N_CORES // B)
        h0 = hg * HEADS_PER_CORE
        in_maps.append(
            {
                "xT": xTs[b],
                "WqS": np.ascontiguousarray(Wq16[:, h0 * HS : (h0 + HEADS_PER_CORE) * HS]),
                "bqS": np.ascontiguousarray(
                    bq[h0 * HS : (h0 + HEADS_PER_CORE) * HS].reshape(2, 128).T
                ),
            }
        )

    res = run_bass_kernel_spmd(nc, in_maps, core_ids=list(range(N_CORES)))

    full = np.empty((B, H, S, S), dtype=np.float32)
    for c in range(N_CORES):
        b, hg = divmod(c, N_CORES // B)
        h0 = hg * HEADS_PER_CORE
        full[b, h0 : h0 + HEADS_PER_CORE] = np.asarray(
            res.results[c]["out"]
        ).astype(np.float32)
    return full


# revision 33
# speedup vs baseline: 1.1766x; 1.1766x over previous
"""Self-attention scores kernel for Trainium2, 8-core SPMD.

Computes softmax((x@Wq+bq) @ (x@Wq+bq)^T / sqrt(64)) per head
(reference reuses the query projection for k, bug-for-bug).

Sharding: 32 (batch, head) pairs split 4-per-core across 8 cores.
Core c handles batch c//4, heads 4*(c%4) .. 4*(c%4)+3.

Per core: project q^T = Wq_slice^T @ x^T (+bias) from fp16 inputs into
a bf16 q^T tile, then per head stream [128, 2048] score row-blocks:
4 matmuls into a 4-bank PSUM tile, one Exp activation (N=2048, f32
accum row-sum) into a bf16 SBUF tile, reciprocal + per-row scale on
DVE, and bf16 DMA back to HBM (2 MiB groups; the last head drains
per-block, alternating the sync/gpsimd rings). Host upcasts bf16->f32.

bf16 output halves the dominant HBM write traffic vs f32; softmax
values are <= 1 so bf16's 7-bit mantissa costs ~3e-3 relative error,
well within tolerance.
"""

import numpy as np

import concourse.bass as bass
import concourse.mybir as mybir
import concourse.tile as tile
from concourse import bacc
from concourse.bass_utils import run_bass_kernel_spmd

B = 2
S = 2048
D = 1024
H = 16
HS = 64
N_CORES = 8
HEADS_PER_CORE = 4  # 2 pairs of 2 heads (pair = 128 partitions)
KK = D // 128  # 8 k-tiles for the projection contraction
NQ = S // 128  # 16 q row-blocks per head
GRP = 4  # row-blocks batched per output DMA (2 MiB)

# Matmul operand dtype for q^T: bf16 runs at full PE rate and halves
# the per-block LDWEIGHTS time in the PSUM-handoff critical loop vs
# fp32r; its 8-bit mantissa adds ~3e-4 rms to the logits, well inside
# tolerance.
MM_DT = mybir.dt.bfloat16
# Projection inputs in fp16: 11-bit mantissa matches fp32r precision
# while halving the input-load bytes.
IN_DT = mybir.dt.float16
# Output path in bf16: softmax values are <= 1; halves HBM writes.
OUT_DT = mybir.dt.bfloat16

F32 = mybir.dt.float32


def _build():
    nc = bacc.Bacc("TRN2", target_bir_lowering=False, debug=False)
    xT = nc.dram_tensor("xT", [D, S], IN_DT, kind="ExternalInput").ap()
    WqS = nc.dram_tensor("WqS", [D, HEADS_PER_CORE * HS], IN_DT, kind="ExternalInput").ap()
    bqS = nc.dram_tensor("bqS", [128, 2], F32, kind="ExternalInput").ap()
    out = nc.dram_tensor("out", [HEADS_PER_CORE, S, S], OUT_DT, kind="ExternalOutput").ap()

    with tile.TileContext(nc) as tc:
        with (
            tc.tile_pool(name="consts", bufs=1) as consts,
            tc.tile_pool(name="xt", bufs=KK) as xt_pool,
            tc.tile_pool(name="ps", bufs=2, space="PSUM") as ps_pool,
            tc.tile_pool(name="et", bufs=3) as et_pool,
            tc.tile_pool(name="small", bufs=4) as small,
        ):
            w = consts.tile([128, KK, HEADS_PER_CORE * HS], IN_DT)
            nc.scalar.dma_start(out=w[:], in_=WqS.rearrange("(kk p) c -> p kk c", p=128))
            bias = consts.tile([128, 2], F32)
            nc.scalar.dma_start(out=bias[:], in_=bqS)

            # x^T streamed as 8 independent k-tiles so projection matmuls
            # can start as soon as each tile lands.
            xts = []
            for kk in range(KK):
                xtt = xt_pool.tile([128, S], IN_DT, tag="xt")
                nc.sync.dma_start(out=xtt[:], in_=xT[kk * 128 : (kk + 1) * 128, :])
                xts.append(xtt)

            # ---- Projection for one head-pair: fills a full 4-bank PSUM
            # tile (4 x 512-col accumulation groups of 8 matmuls), then one
            # bias-add into the persistent q^T tile.
            # Both pairs' projections interleaved kk-outer: PE streams the
            # 8-matmul kk groups as each x^T tile lands, so the projection
            # finishes right after the input load instead of serializing
            # 64 matmuls after it.
            psA = ps_pool.tile([128, S], F32, tag="ps", name="psA")
            psB = ps_pool.tile([128, S], F32, tag="ps", name="psB")
            pss = [psA, psB]
            for kk in range(KK):
                for g in range(2):
                    for n in range(4):
                        nc.tensor.matmul(
                            pss[g][:, n * 512 : (n + 1) * 512],
                            lhsT=w[:, kk, g * 128 : (g + 1) * 128],
                            rhs=xts[kk][:, n * 512 : (n + 1) * 512],
                            start=(kk == 0),
                            stop=(kk == KK - 1),
                        )
            qts = []
            for g in range(2):
                qtg = consts.tile([128, S], MM_DT, tag=f"qt{g}", name=f"qt{g}")
                nc.vector.tensor_scalar_add(qtg[:], pss[g][:], bias[:, g : g + 1])
                qts.append(qtg)

            # ---- Scores + softmax, streamed in groups of 4 row-blocks ----
            # The last head drains per-block (512 KiB DMAs right after each
            # row-scale) so the kernel tail trails the final exp by one
            # block; earlier heads use 2 MiB group DMAs. Output DMAs
            # alternate between the HWDGE (sync) and SWDGE (gpsimd) rings —
            # the Pool engine is otherwise idle, and the second ring adds
            # queue parallelism to the write stream.
            for h in range(HEADS_PER_CORE):
                qtg = qts[h // 2]
                pb = (h % 2) * 64
                for grp in range(NQ // GRP):
                    last_grp = h == HEADS_PER_CORE - 1
                    et = et_pool.tile([128, GRP, S], OUT_DT, tag="et")
                    sums = small.tile([128, GRP], F32, tag="sm")
                    rec = small.tile([128, GRP], F32, tag="rc")
                    for q in range(GRP):
                        i = grp * GRP + q
                        ps = ps_pool.tile([128, S], F32, tag="ps")
                        lhsT = qtg[pb : pb + 64, i * 128 : (i + 1) * 128]
                        for j in range(4):
                            nc.tensor.matmul(
                                ps[:, j * 512 : (j + 1) * 512],
                                lhsT=lhsT,
                                rhs=qtg[pb : pb + 64, j * 512 : (j + 1) * 512],
                                start=True,
                                stop=True,
                            )
                        nc.scalar.activation(
                            out=et[:, q, :],
                            in_=ps[:],
                            func=mybir.ActivationFunctionType.Exp,
                            scale=1.0 / np.sqrt(float(HS)),
                            accum_out=sums[:, q : q + 1],
                        )
                        if last_grp:
                            nc.vector.reciprocal(rec[:, q : q + 1], sums[:, q : q + 1])
                            nc.vector.tensor_scalar_mul(
                                et[:, q, :], et[:, q, :], rec[:, q : q + 1]
                            )
                            eng = nc.sync if i % 2 == 0 else nc.gpsimd
                            eng.dma_start(
                                out=out[h, i * 128 : (i + 1) * 128, :],
                                in_=et[:, q, :],
                            )
                    if last_grp:
                        continue
                    nc.vector.reciprocal(rec[:], sums[:])
                    for q in range(GRP):
                        nc.vector.tensor_scalar_mul(
                            et[:, q, :], et[:, q, :], rec[:, q : q + 1]
                        )
                    eng = nc.sync if grp % 2 == 0 else nc.gpsimd
                    eng.dma_start(
                        out=out[h, grp * GRP * 128 : (grp + 1) * GRP * 128, :].rearrange(
                            "(c p) s -> p c s", p=128
                        ),
                        in_=et[:],
                    )
    nc.compile()
    return nc


_NC_CACHE = None


def kernel(x, Wq, bq):
    global _NC_CACHE
    x = np.asarray(x, dtype=np.float32)
    Wq = np.asarray(Wq, dtype=np.float32)
    bq = np.asarray(bq, dtype=np.float32)
    assert x.shape == (B, S, D) and Wq.shape == (D, D) and bq.shape == (D,)

    if _NC_CACHE is None:
        _NC_CACHE = _build()
    nc = _NC_CACHE

    xTs = [np.ascontiguousarray(x[b].T.astype(np.float16)) for b in range(B)]
    Wq16 = Wq.astype(np.float16)
    in_maps = []
    for c in range(N_CORES):
        b, hg = divmod(c, N_CORES // B)
        h0 = hg * HEADS_PER_CORE
        in_maps.append(
            {
                "xT": xTs[b],
                "WqS": np.ascontiguousarray(Wq16[:, h0 * HS : (h0 + HEADS_PER_CORE) * HS]),
                "bqS": np.ascontiguousarray(
                    bq[h0 * HS : (h0 + HEADS_PER_CORE) * HS].reshape(2, 128).T
                ),
            }
        )

    res = run_bass_kernel_spmd(nc, in_maps, core_ids=list(range(N_CORES)))

    full = np.empty((B, H, S, S), dtype=np.float32)
    for c in range(N_CORES):
        b, hg = divmod(c, N_CORES // B)
        h0 = hg * HEADS_PER_CORE
        full[b, h0 : h0 + HEADS_PER_CORE] = np.asarray(
            res.results[c]["out"]
        ).astype(np.float32)
    return full
